# revision 31
# baseline (speedup 1.0000x reference)
"""Sparse (relu-cosine, causal+padding-masked) attention on 8 TRN2 NeuronCores.

Contract: kernel(**inputs) takes the full unsharded inputs and returns the
full [B, S, D] output. Internally:
  - host: compact each batch's tokens to the mask-valid ones (queries and
    keys share the same validity mask, so causal structure stays exactly
    lower-triangular in compacted space and all masking disappears),
    transpose X, slice per-head-pair weights, pad to tile multiples.
  - device (SPMD, 8 cores, 2 heads per core): QKV projections, cosine
    normalization folded into the relu scale (1/||k||) and a per-query
    broadcast tile (1/||q||), relu(QK^T) with triangular masks only on
    diagonal tiles, context accumulation (col-tiled pairs), and a partial
    output projection (transposed layout) through this core's 128 columns
    of Wo.
  - host: sum the 8 partial outputs, scatter rows back to the full
    [B, S, D] layout (masked query rows are exactly zero).

Matmul operands are bf16; every accumulation (PSUM) is fp32 and the
norm scales (1/||q||, 1/||k||) are computed from the fp32 sums, so the
cosine normalization is exact for the bf16-rounded Q/K. Attention is
software-pipelined per q-block: scores of block i+1 are issued to the PE
before the context matmuls of block i, so the PE never waits on relu.
"""

import numpy as np

B, S, D, H = 2, 2048, 1024, 16
DH = D // H
NCORES = 8
HEADS_PER_CORE = H // NCORES  # 2
NH = HEADS_PER_CORE
JW = HEADS_PER_CORE * DH  # 128, per-core head-dim slice width
QB = 512  # query block width (one fp32 PSUM bank)
KT = 128  # key tile (partition dim)


def _build_program(LQs, n_dblk=D // 128):
    import concourse.bass as bass
    import concourse.mybir as mybir
    import concourse.tile as tile
    from concourse import bacc
    from concourse.bass import ts
    from concourse.masks import make_identity

    F32 = mybir.dt.float32
    F32R = mybir.dt.float32r
    BF16 = mybir.dt.bfloat16
    AF = mybir.ActivationFunctionType
    MULT = mybir.AluOpType.mult
    MAX = mybir.AluOpType.max

    LT = sum(LQs)
    offs = [0, LQs[0]]
    n_ttiles = LT // 128

    nc = bacc.Bacc("TRN2", target_bir_lowering=False, debug=False,
                   num_devices=NCORES)

    XT = nc.dram_tensor("XT", [D, LT], BF16, kind="ExternalInput").ap()
    WQT = nc.dram_tensor("WQT", [D, JW], BF16, kind="ExternalInput").ap()
    WKT = nc.dram_tensor("WKT", [D, JW], BF16, kind="ExternalInput").ap()
    WVT = nc.dram_tensor("WVT", [D, JW], BF16, kind="ExternalInput").ap()
    WOT = nc.dram_tensor("WOT", [JW, D], BF16, kind="ExternalInput").ap()
    # causal diag-tile masks: CAUS[:, 384-off : 384-off+qbw], off = kt0-q0
    CAUS = nc.dram_tensor("CAUS", [128, 896], BF16, kind="ExternalInput").ap()
    # IND[j, h] = 1 if j // DH == h ; INDT is its transpose
    IND = nc.dram_tensor("IND", [JW, NH], BF16, kind="ExternalInput").ap()
    INDT = nc.dram_tensor("INDT", [NH, JW], F32R, kind="ExternalInput").ap()
    # transposed output: OUTT[dout, t]; host transposes back
    OUTT = nc.dram_tensor("OUTT", [D, LT], F32, kind="ExternalOutput").ap()

    EPS = 1e-12

    def col_blocks(width, bw=QB):
        blocks = []
        c = 0
        while c < width:
            w = min(bw, width - c)
            blocks.append((c, w))
            c += w
        return blocks

    with tile.TileContext(nc) as tc:
        with (
            tc.tile_pool(name="consts", bufs=1) as consts,
            tc.tile_pool(name="proj", bufs=1) as projp,
            tc.tile_pool(name="work", bufs=3) as work,
            tc.tile_pool(name="outp", bufs=4) as outp,
            tc.tile_pool(name="ps_mm", bufs=1, space="PSUM") as ps_mm,
            tc.tile_pool(name="ps_scp", bufs=3, space="PSUM") as ps_scp,
            tc.tile_pool(name="ps_ctxp", bufs=1, space="PSUM") as ps_ctxp,
        ):
            # ---- weights first (first projection matmul needs them) --------
            wqt = consts.tile([128, n_dblk, JW], BF16)
            wkt = consts.tile([128, n_dblk, JW], BF16)
            wvt = consts.tile([128, n_dblk, JW], BF16)
            nc.sync.dma_start(out=wqt, in_=WQT.rearrange("(k p) j -> p k j", p=128))
            nc.sync.dma_start(out=wkt, in_=WKT.rearrange("(k p) j -> p k j", p=128))
            nc.sync.dma_start(out=wvt, in_=WVT.rearrange("(k p) j -> p k j", p=128))

            caus = consts.tile([128, 896], BF16)
            nc.sync.dma_start(out=caus, in_=CAUS[:, :])

            xtp = tc.alloc_tile_pool(name="xt", bufs=1)
            # ---- X^T (all d-blocks resident) -------------------------------
            xt = xtp.tile([128, n_dblk, LT], BF16)
            c0 = min(QB, LT)
            for k in range(n_dblk):
                nc.sync.dma_start(out=xt[:, k, :c0], in_=XT[ts(k, 128), :c0])
            for k in range(n_dblk):
                nc.sync.dma_start(out=xt[:, k, c0:], in_=XT[ts(k, 128), c0:])

            # ---- remaining constants ---------------------------------------
            wot = consts.tile([JW, D], BF16)
            nc.sync.dma_start(out=wot, in_=WOT[:, :])
            ind = consts.tile([JW, NH], BF16)
            nc.sync.dma_start(out=ind, in_=IND[:, :])
            indt = consts.tile([NH, JW], F32R)
            nc.sync.dma_start(out=indt, in_=INDT[:, :])
            eps128 = consts.tile([128, 1], F32)
            nc.vector.memset(eps128, EPS)
            ident = consts.tile([128, 128], BF16)
            make_identity(nc, ident)

            # ---- projections ------------------------------------------------
            # qt/kt/vt in bf16; squares for the norms are taken from the f32
            # PSUM so the scales stay exact for the rounded Q/K.
            qt = projp.tile([JW, LT], BF16)
            kt_ = projp.tile([JW, LT], BF16)
            vt = projp.tile([JW, LT], BF16)
            qsq = projp.tile([JW, LT], BF16)
            ksq = projp.tile([JW, LT], BF16)
            cp_i = 0
            for c0, w in col_blocks(LT):
                for dst, wmat, sq in ((qt, wqt, qsq), (kt_, wkt, ksq),
                                      (vt, wvt, None)):
                    ps = ps_scp.tile([JW, QB], F32, tag="sc", name="ps_proj")
                    for k in range(n_dblk):
                        nc.tensor.matmul(
                            ps[:, :w], wmat[:, k, :], xt[:, k, c0:c0 + w],
                            start=(k == 0), stop=(k == n_dblk - 1),
                        )
                    if cp_i % 2 == 0:
                        nc.vector.tensor_copy(dst[:, c0:c0 + w], ps[:, :w])
                    else:
                        nc.scalar.activation(out=dst[:, c0:c0 + w],
                                             in_=ps[:, :w], func=AF.Copy)
                    cp_i += 1
                    if sq is not None:
                        nc.vector.tensor_mul(sq[:, c0:c0 + w],
                                             dst[:, c0:c0 + w],
                                             dst[:, c0:c0 + w])

            xtp.release()
            max_nkt = max(LQs) // KT
            att_bufs = 3 if max_nkt <= 10 else (2 if max_nkt <= 16 else 1)
            attp = tc.alloc_tile_pool(name="att", bufs=att_bufs)

            # ---- kscale[t, h] = rsqrt(sum_j ksq[j, t] over head h) ----------
            ksum_ps = ps_ctxp.tile([128, n_ttiles * NH], F32, tag="ctx_ps",
                                   name="ksum_ps")
            for tt in range(n_ttiles):
                nc.tensor.matmul(ksum_ps[:, tt * NH:(tt + 1) * NH],
                                 ksq[:, ts(tt, 128)], ind[:, :],
                                 start=True, stop=True, skip_group_check=True)
            ksc = projp.tile([128, n_ttiles, NH], F32)
            nc.scalar.activation(out=ksc[:, :, :].rearrange("p a b -> p (a b)"),
                                 in_=ksum_ps[:, :], func=AF.Sqrt,
                                 bias=eps128[:, :], scale=1.0)
            nc.vector.reciprocal(out=ksc[:, :, :].rearrange("p a b -> p (a b)"),
                                 in_=ksc[:, :, :].rearrange("p a b -> p (a b)"))

            # ---- V natural via PE transpose --------------------------------
            vn = projp.tile([128, n_ttiles, JW], BF16)
            for tt in range(n_ttiles):
                ps = ps_scp.tile([128, 128], BF16, tag="sc", name="ps_vtr")
                nc.tensor.transpose(ps[:, :], vt[:, ts(tt, 128)], ident)
                for h in range(NH):
                    if tt % 2 == 0:
                        nc.vector.tensor_scalar_mul(
                            out=vn[:, tt, ts(h, DH)], in0=ps[:, ts(h, DH)],
                            scalar1=ksc[:, tt, h:h + 1])
                    else:
                        nc.scalar.activation(
                            out=vn[:, tt, ts(h, DH)], in_=ps[:, ts(h, DH)],
                            func=AF.Copy, scale=ksc[:, tt, h:h + 1])

            # ---- attention, software-pipelined over q-blocks ----------------
            blocks = []
            for b in range(B):
                for q0, qw in col_blocks(LQs[b]):
                    blocks.append((b, q0, qw))
            ctx_sbs = {
                b: attp.tile([JW, LQs[b]], BF16, tag=f"ctx_{b}", bufs=1,
                             name=f"ctx_sb{b}")
                for b in range(B)
            }

            state = {}
            qsbs = {}

            def emit_qsb(blk):
                # QSB[p, q] = rsqrt(|q|^2 + eps)[q, head(p)] broadcast tile;
                # emitted one block ahead so the PE never waits on the
                # sqrt/reciprocal chain at a block boundary
                b, q0, qw = blk
                ob = offs[b]
                ps_ss = ps_mm.tile([NH, QB], F32, tag="mm", name="ps_qsum")
                nc.tensor.matmul(ps_ss[:, :qw], ind[:, :],
                                 qsq[:, ob + q0:ob + q0 + qw],
                                 start=True, stop=True)
                ssq = work.tile([NH, QB], F32, tag="ssq")
                nc.scalar.activation(out=ssq[:, :qw], in_=ps_ss[:, :qw],
                                     func=AF.Sqrt, bias=eps128[:NH, :],
                                     scale=1.0)
                ssr = work.tile([NH, QB], F32R, tag="ssr")
                with nc.allow_low_precision(reason="only feeds the broadcast "
                                            "matmul; fp32 kept via psum"):
                    nc.vector.reciprocal(out=ssr[:, :qw], in_=ssq[:, :qw])
                ps_qsb = ps_mm.tile([128, QB], F32, tag="mm", name="ps_qsb")
                nc.tensor.matmul(ps_qsb[:, :qw], indt[:, :], ssr[:, :qw],
                                 start=True, stop=True)
                qsb = work.tile([128, QB], F32, tag="qsb")
                nc.vector.tensor_copy(qsb[:, :qw], ps_qsb[:, :qw])
                qsbs[blk] = qsb

            def emit_scores(blk):
                b, q0, qw = blk
                ob = offs[b]
                lq = LQs[b]
                qsb = qsbs.pop(blk)

                n_kt = min((q0 + qw + KT - 1) // KT, lq // KT)
                att_sb = attp.tile([128, max_nkt * NH, QB], BF16,
                                   tag="att_sb", name="att_sb")
                offs_ki = []
                diag_i = 0
                for ki in range(n_kt):
                    k0 = ki * KT
                    # columns < off are fully masked by causality; skip them
                    off = max(0, k0 - q0)
                    offs_ki.append(off)
                    w = qw - off
                    diag = k0 > q0 - KT
                    sc_ps = ps_scp.tile([128, NH, QB], F32, tag="sc",
                                        name="sc_ps")
                    for h in range(NH):
                        nc.tensor.matmul(
                            sc_ps[:, h, off:qw],
                            kt_[ts(h, DH), ob + k0:ob + k0 + KT],
                            qt[ts(h, DH), ob + q0 + off:ob + q0 + qw],
                            start=True, stop=True,
                        )
                    # att = relu(s) for both heads in one op (k-norm scale
                    # lives in V); diagonal tiles fuse the triangular mask:
                    # (s max 0) * caus, with caus broadcast over the head dim
                    sl = ki * NH
                    if diag:
                        cs = caus[:, 384:384 + w]
                        cs2 = bass.AP(tensor=cs.tensor, offset=cs.offset,
                                      ap=[cs.ap[0], [0, NH], cs.ap[1]])
                        if diag_i % 2 == 0:
                            nc.vector.scalar_tensor_tensor(
                                out=att_sb[:, sl:sl + NH, off:qw],
                                in0=sc_ps[:, :, off:qw], scalar=0.0,
                                in1=cs2, op0=MAX, op1=MULT)
                        else:
                            nc.scalar.activation(
                                out=att_sb[:, sl:sl + NH, off:qw],
                                in_=sc_ps[:, :, off:qw], func=AF.Relu)
                            nc.vector.tensor_mul(
                                att_sb[:, sl:sl + NH, off:qw],
                                att_sb[:, sl:sl + NH, off:qw], cs2)
                        diag_i += 1
                    else:
                        nc.scalar.activation(
                            out=att_sb[:, sl:sl + NH, off:qw],
                            in_=sc_ps[:, :, off:qw], func=AF.Relu)
                state[blk] = (att_sb, qsb, n_kt, offs_ki)

            def emit_ctx_out(blk):
                b, q0, qw = blk
                ob = offs[b]
                ctx_sb = ctx_sbs[b]
                att_sb, qsb, n_kt, offs_ki = state.pop(blk)
                # col-tiled pair: both heads accumulate in one PSUM bank
                ctx_ps = ps_ctxp.tile([128, QB], F32, tag="ctx_ps",
                                      name="ctx_ps")
                assert offs_ki[0] == 0  # first tile always starts the bank
                for ki in range(n_kt):
                    gtt = (ob + ki * KT) // KT
                    off = offs_ki[ki]
                    for h in range(NH):
                        nc.tensor.matmul(
                            ctx_ps[ts(h, DH), off:qw],
                            vn[:, gtt, ts(h, DH)],
                            att_sb[:, ki * NH + h, off:qw],
                            start=(ki == 0), stop=(ki == n_kt - 1),
                            tile_position=(0, h * DH),
                            skip_group_check=True,
                        )
                # apply 1/|q| while copying ctx out of PSUM
                nc.vector.tensor_mul(ctx_sb[:, q0:q0 + qw], ctx_ps[:, :qw],
                                     qsb[:, :qw])

                # output projection (transposed layout), this q-block only;
                # dblk pairs share one 2-bank PSUM tile, one copy, one DMA
                for dp in range(n_dblk // 2):
                    ps = ps_scp.tile([128, 2, QB], F32, tag="sc",
                                     name="ps_out")
                    for two in range(2):
                        nc.tensor.matmul(ps[:, two, :qw],
                                         wot[:, ts(dp * 2 + two, 128)],
                                         ctx_sb[:, q0:q0 + qw],
                                         start=True, stop=True)
                    o_sb = outp.tile([128, 2, QB], F32, tag="o_sb")
                    if dp % 2 == 0:
                        nc.vector.tensor_copy(o_sb[:, :, :qw], ps[:, :, :qw])
                    else:
                        nc.scalar.activation(out=o_sb[:, :, :qw],
                                             in_=ps[:, :, :qw], func=AF.Copy)
                    dst = OUTT[dp * 256:(dp + 1) * 256,
                               ob + q0:ob + q0 + qw]
                    nc.sync.dma_start(
                        out=dst.rearrange("(two p) w -> p two w", p=128),
                        in_=o_sb[:, :, :qw])

            emit_qsb(blocks[0])
            for i, blk in enumerate(blocks):
                if i + 1 < len(blocks):
                    emit_qsb(blocks[i + 1])
                emit_scores(blk)
                if i > 0:
                    emit_ctx_out(blocks[i - 1])
            emit_ctx_out(blocks[-1])
            attp.release()

    nc.compile()
    return nc


def _prepare(X, masks, Wq, Wk, Wv, Wo):
    import ml_dtypes
    BF = ml_dtypes.bfloat16

    X = np.asarray(X, dtype=np.float32)
    masks = np.asarray(masks)
    Wq = np.asarray(Wq, dtype=np.float32)
    Wk = np.asarray(Wk, dtype=np.float32)
    Wv = np.asarray(Wv, dtype=np.float32)
    Wo = np.asarray(Wo, dtype=np.float32)

    idxs = [np.where(masks[b] != 0)[0] for b in range(B)]
    LQs = [max(128, int(-(-len(ix) // 128) * 128)) for ix in idxs]
    LT = sum(LQs)
    offs = [0, LQs[0]]

    # compacted, transposed X: columns = valid tokens (zero-padded)
    XTc = np.zeros((D, LT), dtype=np.float32)
    for b in range(B):
        XTc[:, offs[b]:offs[b] + len(idxs[b])] = X[b].T[:, idxs[b]]

    caus = (np.arange(896)[None, :] - 384 >= np.arange(128)[:, None])

    nc = _build_program(LQs)

    in_maps = []
    for c in range(NCORES):
        jsl = slice(c * JW, (c + 1) * JW)
        ind = np.zeros((JW, NH), dtype=np.float32)
        for h in range(NH):
            ind[h * DH:(h + 1) * DH, h] = 1.0
        in_maps.append({
            "XT": XTc.astype(BF),
            "WQT": np.ascontiguousarray(Wq[jsl, :].T).astype(BF),
            "WKT": np.ascontiguousarray(Wk[jsl, :].T).astype(BF),
            "WVT": np.ascontiguousarray(Wv[jsl, :].T).astype(BF),
            "WOT": np.ascontiguousarray(Wo[:, jsl].T).astype(BF),
            "CAUS": caus.astype(BF),
            "IND": ind.astype(BF),
            "INDT": np.ascontiguousarray(ind.T),
        })

    return nc, in_maps, (idxs, LQs, LT, offs)


def _unshard(results, meta):
    idxs, LQs, LT, offs = meta
    partial = np.zeros((D, LT), dtype=np.float64)
    for c in range(NCORES):
        partial += results[c]["OUTT"].astype(np.float64)
    partial = partial.T  # [LT, D]

    out = np.zeros((B, S, D), dtype=np.float32)
    for b in range(B):
        out[b, idxs[b], :] = partial[offs[b]:offs[b] + len(idxs[b]), :].astype(
            np.float32)
    return out


def kernel(X, masks, Wq, Wk, Wv, Wo):
    from concourse.bass_utils import run_bass_kernel_spmd

    nc, in_maps, meta = _prepare(X, masks, Wq, Wk, Wv, Wo)
    res = run_bass_kernel_spmd(nc, in_maps, list(range(NCORES)))
    return _unshard(res.results, meta)


def profile_run(inputs, tmpdir=None):
    """Used by test.py: same program, run with NTFF tracing enabled."""
    from concourse.bass_utils import run_bass_kernel_spmd

    nc, in_maps, meta = _prepare(**inputs)
    res = run_bass_kernel_spmd(nc, in_maps, list(range(NCORES)), trace=True,
                               tmpdir=tmpdir)
    res.output = _unshard(res.results, meta)
    return res


# revision 32
# speedup vs baseline: 1.0460x; 1.0460x over previous
"""Sparse (relu-cosine, causal+padding-masked) attention on 8 TRN2 NeuronCores.

Contract: kernel(**inputs) takes the full unsharded inputs and returns the
full [B, S, D] output. Internally:
  - host: compact each batch's tokens to the mask-valid ones (queries and
    keys share the same validity mask, so causal structure stays exactly
    lower-triangular in compacted space and all masking disappears),
    transpose X, slice per-head-pair weights, pad to tile multiples.
  - device (SPMD, 8 cores, 2 heads per core): QKV projections, cosine
    normalization folded into the relu scale (1/||k||) and a per-query
    broadcast tile (1/||q||), relu(QK^T) with triangular masks only on
    diagonal tiles, context accumulation (col-tiled pairs), and a partial
    output projection (transposed layout) through this core's 128 columns
    of Wo.
  - host: sum the 8 partial outputs, scatter rows back to the full
    [B, S, D] layout (masked query rows are exactly zero).

Matmul operands are bf16; every accumulation (PSUM) is fp32 and the
norm scales (1/||q||, 1/||k||) are computed from the fp32 sums, so the
cosine normalization is exact for the bf16-rounded Q/K. Attention is
software-pipelined per q-block: scores of block i+1 are issued to the PE
before the context matmuls of block i, so the PE never waits on relu.
"""

import numpy as np

B, S, D, H = 2, 2048, 1024, 16
DH = D // H
NCORES = 8
HEADS_PER_CORE = H // NCORES  # 2
NH = HEADS_PER_CORE
JW = HEADS_PER_CORE * DH  # 128, per-core head-dim slice width
QB = 512  # query block width (one fp32 PSUM bank)
KT = 128  # key tile (partition dim)


def _build_program(LQs, n_dblk=D // 128):
    import concourse.bass as bass
    import concourse.mybir as mybir
    import concourse.tile as tile
    from concourse import bacc
    from concourse.bass import ts
    from concourse.masks import make_identity

    F32 = mybir.dt.float32
    F32R = mybir.dt.float32r
    BF16 = mybir.dt.bfloat16
    AF = mybir.ActivationFunctionType
    MULT = mybir.AluOpType.mult
    MAX = mybir.AluOpType.max

    LT = sum(LQs)
    offs = [0, LQs[0]]
    n_ttiles = LT // 128

    nc = bacc.Bacc("TRN2", target_bir_lowering=False, debug=False,
                   num_devices=NCORES)

    XT = nc.dram_tensor("XT", [D, LT], BF16, kind="ExternalInput").ap()
    WQT = nc.dram_tensor("WQT", [D, JW], BF16, kind="ExternalInput").ap()
    WKT = nc.dram_tensor("WKT", [D, JW], BF16, kind="ExternalInput").ap()
    WVT = nc.dram_tensor("WVT", [D, JW], BF16, kind="ExternalInput").ap()
    WOT = nc.dram_tensor("WOT", [JW, D], BF16, kind="ExternalInput").ap()
    # causal diag-tile masks: CAUS[:, 384-off : 384-off+qbw], off = kt0-q0
    CAUS = nc.dram_tensor("CAUS", [128, 896], BF16, kind="ExternalInput").ap()
    # IND[j, h] = 1 if j // DH == h ; INDT is its transpose
    IND = nc.dram_tensor("IND", [JW, NH], BF16, kind="ExternalInput").ap()
    INDT = nc.dram_tensor("INDT", [NH, JW], F32R, kind="ExternalInput").ap()
    # transposed output: OUTT[dout, t]; host transposes back
    OUTT = nc.dram_tensor("OUTT", [D, LT], F32, kind="ExternalOutput").ap()

    EPS = 1e-12

    def col_blocks(width, bw=QB):
        blocks = []
        c = 0
        while c < width:
            w = min(bw, width - c)
            blocks.append((c, w))
            c += w
        return blocks

    with tile.TileContext(nc) as tc:
        with (
            tc.tile_pool(name="consts", bufs=1) as consts,
            tc.tile_pool(name="proj", bufs=1) as projp,
            tc.tile_pool(name="work", bufs=3) as work,
            tc.tile_pool(name="outp", bufs=4) as outp,
            tc.tile_pool(name="ps_mm", bufs=1, space="PSUM") as ps_mm,
            tc.tile_pool(name="ps_scp", bufs=3, space="PSUM") as ps_scp,
            tc.tile_pool(name="ps_ctxp", bufs=1, space="PSUM") as ps_ctxp,
        ):
            # ---- weights first (first projection matmul needs them) --------
            wqt = consts.tile([128, n_dblk, JW], BF16)
            wkt = consts.tile([128, n_dblk, JW], BF16)
            wvt = consts.tile([128, n_dblk, JW], BF16)
            nc.sync.dma_start(out=wqt, in_=WQT.rearrange("(k p) j -> p k j", p=128))
            nc.sync.dma_start(out=wkt, in_=WKT.rearrange("(k p) j -> p k j", p=128))
            nc.sync.dma_start(out=wvt, in_=WVT.rearrange("(k p) j -> p k j", p=128))

            caus = consts.tile([128, 896], BF16)
            nc.sync.dma_start(out=caus, in_=CAUS[:, :])

            xtp = tc.alloc_tile_pool(name="xt", bufs=1)
            # ---- X^T (all d-blocks resident) -------------------------------
            xt = xtp.tile([128, n_dblk, LT], BF16)
            lhalf = min(1024, LT)
            for k in range(n_dblk):
                nc.sync.dma_start(out=xt[:, k, :lhalf],
                                  in_=XT[ts(k, 128), :lhalf])
            for k in range(n_dblk):
                nc.sync.dma_start(out=xt[:, k, lhalf:],
                                  in_=XT[ts(k, 128), lhalf:])

            # ---- remaining constants ---------------------------------------
            wot = consts.tile([JW, D], BF16)
            nc.sync.dma_start(out=wot, in_=WOT[:, :])
            ind = consts.tile([JW, NH], BF16)
            nc.sync.dma_start(out=ind, in_=IND[:, :])
            indt = consts.tile([NH, JW], F32R)
            nc.sync.dma_start(out=indt, in_=INDT[:, :])
            eps128 = consts.tile([128, 1], F32)
            nc.vector.memset(eps128, EPS)
            ident = consts.tile([128, 128], BF16)
            make_identity(nc, ident)

            # ---- projections ------------------------------------------------
            # qt/kt/vt in bf16; squares for the norms are taken from the f32
            # PSUM so the scales stay exact for the rounded Q/K.
            qt = projp.tile([JW, LT], BF16)
            kt_ = projp.tile([JW, LT], BF16)
            vt = projp.tile([JW, LT], BF16)
            qsq = projp.tile([JW, LT], BF16)
            ksq = projp.tile([JW, LT], BF16)
            cp_i = 0
            for c0, w in col_blocks(LT):
                for dst, wmat, sq in ((qt, wqt, qsq), (kt_, wkt, ksq),
                                      (vt, wvt, None)):
                    ps = ps_scp.tile([JW, QB], F32, tag="sc", name="ps_proj")
                    for k in range(n_dblk):
                        nc.tensor.matmul(
                            ps[:, :w], wmat[:, k, :], xt[:, k, c0:c0 + w],
                            start=(k == 0), stop=(k == n_dblk - 1),
                        )
                    if cp_i % 2 == 0:
                        nc.vector.tensor_copy(dst[:, c0:c0 + w], ps[:, :w])
                    else:
                        nc.scalar.activation(out=dst[:, c0:c0 + w],
                                             in_=ps[:, :w], func=AF.Copy)
                    cp_i += 1
                    if sq is not None:
                        nc.vector.tensor_mul(sq[:, c0:c0 + w],
                                             dst[:, c0:c0 + w],
                                             dst[:, c0:c0 + w])

            xtp.release()
            max_nkt = max(LQs) // KT
            att_bufs = 3 if max_nkt <= 10 else (2 if max_nkt <= 16 else 1)
            attp = tc.alloc_tile_pool(name="att", bufs=att_bufs)

            # ---- kscale[t, h] = rsqrt(sum_j ksq[j, t] over head h) ----------
            ksum_ps = ps_ctxp.tile([128, n_ttiles * NH], F32, tag="ctx_ps",
                                   name="ksum_ps")
            for tt in range(n_ttiles):
                nc.tensor.matmul(ksum_ps[:, tt * NH:(tt + 1) * NH],
                                 ksq[:, ts(tt, 128)], ind[:, :],
                                 start=True, stop=True, skip_group_check=True)
            ksc = projp.tile([128, n_ttiles, NH], F32)
            nc.scalar.activation(out=ksc[:, :, :].rearrange("p a b -> p (a b)"),
                                 in_=ksum_ps[:, :], func=AF.Sqrt,
                                 bias=eps128[:, :], scale=1.0)
            nc.vector.reciprocal(out=ksc[:, :, :].rearrange("p a b -> p (a b)"),
                                 in_=ksc[:, :, :].rearrange("p a b -> p (a b)"))

            # ---- V natural via PE transpose --------------------------------
            vn = projp.tile([128, n_ttiles, JW], BF16)
            for tt in range(n_ttiles):
                ps = ps_scp.tile([128, 128], BF16, tag="sc", name="ps_vtr")
                nc.tensor.transpose(ps[:, :], vt[:, ts(tt, 128)], ident)
                for h in range(NH):
                    if tt % 2 == 0:
                        nc.vector.tensor_scalar_mul(
                            out=vn[:, tt, ts(h, DH)], in0=ps[:, ts(h, DH)],
                            scalar1=ksc[:, tt, h:h + 1])
                    else:
                        nc.scalar.activation(
                            out=vn[:, tt, ts(h, DH)], in_=ps[:, ts(h, DH)],
                            func=AF.Copy, scale=ksc[:, tt, h:h + 1])

            # ---- attention, software-pipelined over q-blocks ----------------
            blocks = []
            for b in range(B):
                for q0, qw in col_blocks(LQs[b]):
                    blocks.append((b, q0, qw))
            ctx_sbs = {
                b: attp.tile([JW, LQs[b]], BF16, tag=f"ctx_{b}", bufs=1,
                             name=f"ctx_sb{b}")
                for b in range(B)
            }

            state = {}
            qsbs = {}

            def emit_qsb(blk):
                # QSB[p, q] = rsqrt(|q|^2 + eps)[q, head(p)] broadcast tile;
                # emitted one block ahead so the PE never waits on the
                # sqrt/reciprocal chain at a block boundary
                b, q0, qw = blk
                ob = offs[b]
                ps_ss = ps_mm.tile([NH, QB], F32, tag="mm", name="ps_qsum")
                nc.tensor.matmul(ps_ss[:, :qw], ind[:, :],
                                 qsq[:, ob + q0:ob + q0 + qw],
                                 start=True, stop=True)
                ssq = work.tile([NH, QB], F32, tag="ssq")
                nc.scalar.activation(out=ssq[:, :qw], in_=ps_ss[:, :qw],
                                     func=AF.Sqrt, bias=eps128[:NH, :],
                                     scale=1.0)
                ssr = work.tile([NH, QB], F32R, tag="ssr")
                with nc.allow_low_precision(reason="only feeds the broadcast "
                                            "matmul; fp32 kept via psum"):
                    nc.vector.reciprocal(out=ssr[:, :qw], in_=ssq[:, :qw])
                ps_qsb = ps_mm.tile([128, QB], F32, tag="mm", name="ps_qsb")
                nc.tensor.matmul(ps_qsb[:, :qw], indt[:, :], ssr[:, :qw],
                                 start=True, stop=True)
                qsb = work.tile([128, QB], F32, tag="qsb")
                nc.vector.tensor_copy(qsb[:, :qw], ps_qsb[:, :qw])
                qsbs[blk] = qsb

            def emit_scores(blk):
                b, q0, qw = blk
                ob = offs[b]
                lq = LQs[b]
                qsb = qsbs.pop(blk)

                n_kt = min((q0 + qw + KT - 1) // KT, lq // KT)
                att_sb = attp.tile([128, max_nkt * NH, QB], BF16,
                                   tag="att_sb", name="att_sb")
                offs_ki = []
                diag_i = 0
                for ki in range(n_kt):
                    k0 = ki * KT
                    # columns < off are fully masked by causality; skip them
                    off = max(0, k0 - q0)
                    offs_ki.append(off)
                    w = qw - off
                    diag = k0 > q0 - KT
                    sc_ps = ps_scp.tile([128, NH, QB], F32, tag="sc",
                                        name="sc_ps")
                    for h in range(NH):
                        nc.tensor.matmul(
                            sc_ps[:, h, off:qw],
                            kt_[ts(h, DH), ob + k0:ob + k0 + KT],
                            qt[ts(h, DH), ob + q0 + off:ob + q0 + qw],
                            start=True, stop=True,
                        )
                    # att = relu(s) for both heads in one op (k-norm scale
                    # lives in V); diagonal tiles fuse the triangular mask:
                    # (s max 0) * caus, with caus broadcast over the head dim
                    sl = ki * NH
                    if diag:
                        cs = caus[:, 384:384 + w]
                        cs2 = bass.AP(tensor=cs.tensor, offset=cs.offset,
                                      ap=[cs.ap[0], [0, NH], cs.ap[1]])
                        if diag_i % 2 == 0:
                            nc.vector.scalar_tensor_tensor(
                                out=att_sb[:, sl:sl + NH, off:qw],
                                in0=sc_ps[:, :, off:qw], scalar=0.0,
                                in1=cs2, op0=MAX, op1=MULT)
                        else:
                            nc.scalar.activation(
                                out=att_sb[:, sl:sl + NH, off:qw],
                                in_=sc_ps[:, :, off:qw], func=AF.Relu)
                            nc.vector.tensor_mul(
                                att_sb[:, sl:sl + NH, off:qw],
                                att_sb[:, sl:sl + NH, off:qw], cs2)
                        diag_i += 1
                    else:
                        nc.scalar.activation(
                            out=att_sb[:, sl:sl + NH, off:qw],
                            in_=sc_ps[:, :, off:qw], func=AF.Relu)
                state[blk] = (att_sb, qsb, n_kt, offs_ki)

            def emit_ctx_out(blk):
                b, q0, qw = blk
                ob = offs[b]
                ctx_sb = ctx_sbs[b]
                att_sb, qsb, n_kt, offs_ki = state.pop(blk)
                # col-tiled pair: both heads accumulate in one PSUM bank
                ctx_ps = ps_ctxp.tile([128, QB], F32, tag="ctx_ps",
                                      name="ctx_ps")
                assert offs_ki[0] == 0  # first tile always starts the bank
                for ki in range(n_kt):
                    gtt = (ob + ki * KT) // KT
                    off = offs_ki[ki]
                    for h in range(NH):
                        nc.tensor.matmul(
                            ctx_ps[ts(h, DH), off:qw],
                            vn[:, gtt, ts(h, DH)],
                            att_sb[:, ki * NH + h, off:qw],
                            start=(ki == 0), stop=(ki == n_kt - 1),
                            tile_position=(0, h * DH),
                            skip_group_check=True,
                        )
                # apply 1/|q| while copying ctx out of PSUM
                nc.vector.tensor_mul(ctx_sb[:, q0:q0 + qw], ctx_ps[:, :qw],
                                     qsb[:, :qw])

                # output projection (transposed layout), this q-block only;
                # dblk pairs share one 2-bank PSUM tile, one copy, one DMA
                for dp in range(n_dblk // 2):
                    ps = ps_scp.tile([128, 2, QB], F32, tag="sc",
                                     name="ps_out")
                    for two in range(2):
                        nc.tensor.matmul(ps[:, two, :qw],
                                         wot[:, ts(dp * 2 + two, 128)],
                                         ctx_sb[:, q0:q0 + qw],
                                         start=True, stop=True)
                    o_sb = outp.tile([128, 2, QB], F32, tag="o_sb")
                    if dp % 2 == 0:
                        nc.vector.tensor_copy(o_sb[:, :, :qw], ps[:, :, :qw])
                    else:
                        nc.scalar.activation(out=o_sb[:, :, :qw],
                                             in_=ps[:, :, :qw], func=AF.Copy)
                    dst = OUTT[dp * 256:(dp + 1) * 256,
                               ob + q0:ob + q0 + qw]
                    nc.sync.dma_start(
                        out=dst.rearrange("(two p) w -> p two w", p=128),
                        in_=o_sb[:, :, :qw])

            emit_qsb(blocks[0])
            for i, blk in enumerate(blocks):
                if i + 1 < len(blocks):
                    emit_qsb(blocks[i + 1])
                emit_scores(blk)
                if i > 0:
                    emit_ctx_out(blocks[i - 1])
            emit_ctx_out(blocks[-1])
            attp.release()

    nc.compile()
    return nc


def _prepare(X, masks, Wq, Wk, Wv, Wo):
    import ml_dtypes
    BF = ml_dtypes.bfloat16

    X = np.asarray(X, dtype=np.float32)
    masks = np.asarray(masks)
    Wq = np.asarray(Wq, dtype=np.float32)
    Wk = np.asarray(Wk, dtype=np.float32)
    Wv = np.asarray(Wv, dtype=np.float32)
    Wo = np.asarray(Wo, dtype=np.float32)

    idxs = [np.where(masks[b] != 0)[0] for b in range(B)]
    LQs = [max(128, int(-(-len(ix) // 128) * 128)) for ix in idxs]
    LT = sum(LQs)
    offs = [0, LQs[0]]

    # compacted, transposed X: columns = valid tokens (zero-padded)
    XTc = np.zeros((D, LT), dtype=np.float32)
    for b in range(B):
        XTc[:, offs[b]:offs[b] + len(idxs[b])] = X[b].T[:, idxs[b]]

    caus = (np.arange(896)[None, :] - 384 >= np.arange(128)[:, None])

    nc = _build_program(LQs)

    in_maps = []
    for c in range(NCORES):
        jsl = slice(c * JW, (c + 1) * JW)
        ind = np.zeros((JW, NH), dtype=np.float32)
        for h in range(NH):
            ind[h * DH:(h + 1) * DH, h] = 1.0
        in_maps.append({
            "XT": XTc.astype(BF),
            "WQT": np.ascontiguousarray(Wq[jsl, :].T).astype(BF),
            "WKT": np.ascontiguousarray(Wk[jsl, :].T).astype(BF),
            "WVT": np.ascontiguousarray(Wv[jsl, :].T).astype(BF),
            "WOT": np.ascontiguousarray(Wo[:, jsl].T).astype(BF),
            "CAUS": caus.astype(BF),
            "IND": ind.astype(BF),
            "INDT": np.ascontiguousarray(ind.T),
        })

    return nc, in_maps, (idxs, LQs, LT, offs)


def _unshard(results, meta):
    idxs, LQs, LT, offs = meta
    partial = np.zeros((D, LT), dtype=np.float64)
    for c in range(NCORES):
        partial += results[c]["OUTT"].astype(np.float64)
    partial = partial.T  # [LT, D]

    out = np.zeros((B, S, D), dtype=np.float32)
    for b in range(B):
        out[b, idxs[b], :] = partial[offs[b]:offs[b] + len(idxs[b]), :].astype(
            np.float32)
    return out


def kernel(X, masks, Wq, Wk, Wv, Wo):
    from concourse.bass_utils import run_bass_kernel_spmd

    nc, in_maps, meta = _prepare(X, masks, Wq, Wk, Wv, Wo)
    res = run_bass_kernel_spmd(nc, in_maps, list(range(NCORES)))
    return _unshard(res.results, meta)


def profile_run(inputs, tmpdir=None):
    """Used by test.py: same program, run with NTFF tracing enabled."""
    from concourse.bass_utils import run_bass_kernel_spmd

    nc, in_maps, meta = _prepare(**inputs)
    res = run_bass_kernel_spmd(nc, in_maps, list(range(NCORES)), trace=True,
                               tmpdir=tmpdir)
    res.output = _unshard(res.results, meta)
    return res
